# revision 13
# baseline (speedup 1.0000x reference)
"""Distributed Trainium2 (Bass/Tile) kernel for a batched quantized matmul.

Reference computation (all shapes hardcoded):
    out[s,b,m,n] = sum_k (x[s,b,m,k] + 66)*0.03 * (y[b,k,n] - 160)*0.025
    x: [7, 8, 1024, 1024] f32 holding ints in [-128, 127]
    y: [8, 1024, 1024]    f32 holding ints in [0, 255]
    out: [7, 8, 1024, 1024] f32

Sharding: data-parallel over B=8 -> one batch element b per NeuronCore.
Core b gets x[:, b] and y[b]; no collectives needed.

Device kernel (per core):
  - Operand values are small integers, so bf16 is EXACT for (x+66) and
    (y-160); the TensorEngine runs at full bf16 rate with fp32 PSUM
    accumulation, matching the f32 reference to ~1e-6.
  - x arrives [m, k]-major but the PE needs the contraction dim K on
    partitions for both operands; the host shards x in [k, m]-major
    layout (pure layout transform) so the device does contiguous
    full-bandwidth DMA loads. (The DMA xbar transpose path hits a
    walrus "Too many sync wait commands" limit under Tile.)
  - Zero-points applied on device: -160 on y via ScalarE activation,
    +66 on xT via VectorE tensor_scalar; the combined scale
    0.03*0.025 = 7.5e-4 is fused into the PSUM->SBUF eviction copy.
"""

import numpy as np
import ml_dtypes

import concourse.bass as bass
import concourse.mybir as mybir
from concourse import bacc
from concourse.tile import TileContext
from concourse.bass_utils import run_bass_kernel_spmd

S, B, M, K, N = 7, 8, 1024, 1024, 1024
P = 128          # SBUF partitions / PE array dim
NB = 512         # one PSUM bank of fp32
X_ZP = -66.0
Y_ZP = 160.0
OUT_SCALE = 0.03 * 0.025
BF16 = mybir.dt.bfloat16
F32 = mybir.dt.float32
ACT_COPY = mybir.ActivationFunctionType.Copy

_CACHED_NC = None


def build():
    # Bacc (not plain Bass): its finalize() runs generate_event_semaphores,
    # which splits multi-wait sync_info to the <=1-wait-per-instruction HW
    # limit (walrus rejects the unsplit form with "Too many sync waits").
    nc = bacc.Bacc("TRN2", target_bir_lowering=False)
    # x is provided k-major per s: xT[s] = x[s].T, shape [S, K, M]
    x_d = nc.declare_dram_parameter("x", [S, K, M], BF16, isOutput=False)
    y_d = nc.declare_dram_parameter("y", [K, N], BF16, isOutput=False)
    o_d = nc.declare_dram_parameter("out", [S, M, N], F32, isOutput=True)
    KT, MT, NT = K // P, M // P, N // NB  # 8, 8, 2

    with TileContext(nc) as tc:
        with tc.tile_pool(name="ypool", bufs=1) as ypool, \
             tc.tile_pool(name="xpool", bufs=2 * KT) as xpool, \
             tc.tile_pool(name="pspool", bufs=8, space="PSUM") as pspool, \
             tc.tile_pool(name="opool", bufs=12) as opool:
            # y[k, n] is already contraction-major; dequant once, keep
            # resident. Interleave y with the s=0 x loads and split each
            # pair across HWDGE (sync) and SWDGE (gpsimd) so the startup
            # fill uses both DGE paths at once — the first matmul group
            # consumes ki chunks in order, so arrival order matters.
            yq = []
            for ki in range(KT):
                yt = ypool.tile([P, N], BF16, tag=f"y{ki}")
                nc.gpsimd.dma_start(out=yt[:], in_=y_d[ki * P:(ki + 1) * P, :])
                nc.scalar.activation(yt[:], yt[:], ACT_COPY, bias=-Y_ZP)
                yq.append(yt)

            for s in range(S):
                # Load x[s] (already [K, M] in DRAM): k-partition strips [P, M].
                xT = []
                for ki in range(KT):
                    xt = xpool.tile([P, M], BF16, tag="xT")
                    eng = nc.sync if ki % 2 == 0 else nc.gpsimd
                    eng.dma_start(
                        out=xt[:], in_=x_d[s, ki * P:(ki + 1) * P, :])
                    nc.vector.tensor_scalar_add(xt[:], xt[:], -X_ZP)
                    xT.append(xt)

                for mj in range(MT):
                    pss = [pspool.tile([P, NB], F32, tag="ps", name=f"ps{nj}")
                           for nj in range(NT)]
                    for ki in range(KT):
                        lhsT = xT[ki][:, mj * P:(mj + 1) * P]
                        for nj in range(NT):
                            nc.tensor.matmul(
                                pss[nj][:], lhsT,
                                yq[ki][:, nj * NB:(nj + 1) * NB],
                                start=(ki == 0), stop=(ki == KT - 1))
                    for nj in range(NT):
                        ot = opool.tile([P, NB], F32, tag="o")
                        nc.scalar.activation(ot[:], pss[nj][:], ACT_COPY,
                                             scale=OUT_SCALE)
                        nc.scalar.dma_start(
                            out=o_d[s, mj * P:(mj + 1) * P,
                                    nj * NB:(nj + 1) * NB],
                            in_=ot[:])
    nc.finalize()
    return nc


def _shard_inputs(x, y):
    bf = ml_dtypes.bfloat16
    in_maps = []
    for b in range(B):
        in_maps.append({
            # all values are integers |v| <= 255 -> bf16 cast is exact;
            # x shard is laid out k-major ([S, K, M]) for the PE
            "x": np.ascontiguousarray(x[:, b].transpose(0, 2, 1)).astype(bf),
            "y": np.ascontiguousarray(y[b]).astype(bf),
        })
    return in_maps


def run(x, y, trace=False):
    global _CACHED_NC
    if _CACHED_NC is None:
        _CACHED_NC = build()
    nc = _CACHED_NC
    in_maps = _shard_inputs(x, y)
    res = run_bass_kernel_spmd(nc, in_maps, core_ids=list(range(B)), trace=trace)
    out = np.stack([np.asarray(res.results[b]["out"]) for b in range(B)], axis=1)
    return out.astype(np.float32), res


def kernel(x, y):
    out, _ = run(x, y, trace=False)
    return out
